# revision 5
# baseline (speedup 1.0000x reference)
"""Cross-attention kernel for 8 TRN2 NeuronCores.

Problem: B=4, Nq=1024, Nkv=4096, C=768, H=12 heads, hd=64.

Sharding: by global (head, batch) block index g = h*4 + b.  Core i owns the 6
blocks g in [6i, 6i+6).  The reference's "scrambling" output reshape
([1,B,H,Nq,hd] -> transpose -> [B*Nq, C]) makes row r of the final projection
input consist of chunks 12r..12r+11 of the flat (h,b,q)-ordered attention
output -- so core i's 6 blocks are exactly the data needed for output rows
[512i, 512(i+1)): zero cross-core communication.

Per core, per block: Q/K/V projections in bf16 (Q^T and K^T in [hd, n]
layout, V natural with a fused ones-column for softmax row sums), scores
S^T = K^T.T @ Q^T per 128-row k-tile, exp on ScalarE (scale=hd^-0.5 folded
in, no max subtraction -- logits are ~N(0,1) so exp is safe), PV accumulated
over k-tiles into PSUM producing O^T [65, 1024] (row 64 = softmax denom).
Normalized O^T goes into OT_dup [128, 6144] (rows 64:128 = copy shifted left
by one column) so the final projection contracts over chunk-pairs (u,u+1)
with full K=128 matmuls reading stride-12 column APs.
"""

import os
import sys
from contextlib import ExitStack

import numpy as np
import ml_dtypes

for _p in ("/opt/trn_rl_repo", "/root/.axon_site/_ro/trn_rl_repo"):
    if _p not in sys.path and os.path.isdir(_p):
        sys.path.append(_p)

import concourse.bass as bass
import concourse.mybir as mybir
import concourse.tile as tile
from concourse import bacc
from concourse.bass_utils import run_bass_kernel_spmd

B, NQ, NK, C, H, HD = 4, 1024, 4096, 768, 12, 64
NCT = C // 128          # 6 contraction tiles of 128
NKT = NK // 128         # 32 k-tiles
NB = 6                  # blocks per core
SLOT_BLOCKS = [[0, 4], [1, 5], [2], [3]]  # local block ids per b-slot
SCALE = HD ** -0.5

f32 = mybir.dt.float32
bf16 = mybir.dt.bfloat16
EXP = mybir.ActivationFunctionType.Exp

_CACHE = {}


def build_nc():
    nc = bacc.Bacc()
    xq_t = nc.declare_dram_parameter("xq_t", [4, C, NQ], bf16, isOutput=False)
    xkv_t = nc.declare_dram_parameter("xkv_t", [4, C, NK], bf16, isOutput=False)
    wqp = nc.declare_dram_parameter("wqp", [4, C, 128], bf16, isOutput=False)
    wkp = nc.declare_dram_parameter("wkp", [4, C, 128], bf16, isOutput=False)
    wvp = nc.declare_dram_parameter("wvp", [4, C, 128], bf16, isOutput=False)
    wp = nc.declare_dram_parameter("wp", [C, C], bf16, isOutput=False)
    bpv = nc.declare_dram_parameter("bpv", [1, C], bf16, isOutput=False)
    out = nc.declare_dram_parameter("out", [512, C], f32, isOutput=True)

    with tile.TileContext(nc) as tc, ExitStack() as ctx:
        consts = ctx.enter_context(tc.tile_pool(name="consts", bufs=1))
        wpool = ctx.enter_context(tc.tile_pool(name="wpool", bufs=1))
        xkvp = ctx.enter_context(tc.tile_pool(name="xkvp", bufs=1))
        xqp = ctx.enter_context(tc.tile_pool(name="xqp", bufs=1))
        ktp = ctx.enter_context(tc.tile_pool(name="ktp", bufs=2))
        v1p = ctx.enter_context(tc.tile_pool(name="v1p", bufs=2))
        qtp = ctx.enter_context(tc.tile_pool(name="qtp", bufs=2))
        ptp = ctx.enter_context(tc.tile_pool(name="ptp", bufs=4))
        otp = ctx.enter_context(tc.tile_pool(name="otp", bufs=1))
        sp = ctx.enter_context(tc.tile_pool(name="sp", bufs=2))
        outp = ctx.enter_context(tc.tile_pool(name="outp", bufs=2))
        psp = ctx.enter_context(tc.tile_pool(name="psp", bufs=2, space="PSUM"))
        pss = ctx.enter_context(tc.tile_pool(name="pss", bufs=2, space="PSUM"))
        pso = ctx.enter_context(tc.tile_pool(name="pso", bufs=1, space="PSUM"))

        # ---- weights / constants (loaded once) ----
        wp_sb = wpool.tile([128, NCT, C], bf16, name="wp_sb")
        nc.sync.dma_start(out=wp_sb, in_=wp.rearrange("(t p) n -> p t n", p=128))
        wq_sb = wpool.tile([128, 4, NCT, 128], bf16, name="wq_sb")
        nc.sync.dma_start(out=wq_sb, in_=wqp.rearrange("s (t p) m -> p s t m", p=128))
        wk_sb = wpool.tile([128, 4, NCT, 128], bf16, name="wk_sb")
        nc.sync.dma_start(out=wk_sb, in_=wkp.rearrange("s (t p) m -> p s t m", p=128))
        wv_sb = wpool.tile([128, 4, NCT, 128], bf16, name="wv_sb")
        nc.sync.dma_start(out=wv_sb, in_=wvp.rearrange("s (t p) m -> p s t m", p=128))
        bp_sb = consts.tile([1, C], bf16, name="bp_sb")
        nc.sync.dma_start(out=bp_sb, in_=bpv[:])
        ones_row = consts.tile([1, 128], bf16, name="ones_row")
        nc.vector.memset(ones_row, 1.0)
        # attention output, chunk-major: col 12*j_row + u;  rows 64:128 shifted
        ot_dup = otp.tile([128, NB * NQ], bf16, name="ot_dup")

        for s in range(4):
            js = SLOT_BLOCKS[s]
            npair = len(js)
            xkv_sb = xkvp.tile([128, NCT, NK], bf16, tag="xkv", name=f"xkv{s}")
            src = xkv_t[s].rearrange("(t p) n -> p t n", p=128)
            for ct in range(NCT):
                nc.sync.dma_start(out=xkv_sb[:, ct, :], in_=src[:, ct, :])
            xq_sb = xqp.tile([128, NCT, NQ], bf16, tag="xq", name=f"xq{s}")
            nc.sync.dma_start(
                out=xq_sb, in_=xq_t[s].rearrange("(t p) n -> p t n", p=128)
            )

            # ---- K^T projection: [64, NK] per block (pair-packed M=128) ----
            kts = [
                ktp.tile([64, NK], bf16, tag="kt", name=f"kt{s}_{i}")
                for i in range(npair)
            ]
            for n in range(NK // 512):
                ps = psp.tile([128, 512], f32, tag="proj", name=f"psk{s}_{n}")
                for ct in range(NCT):
                    nc.tensor.matmul(
                        ps,
                        lhsT=wk_sb[:, s, ct, :],
                        rhs=xkv_sb[:, ct, n * 512 : (n + 1) * 512],
                        start=(ct == 0),
                        stop=(ct == NCT - 1),
                    )
                for i in range(npair):
                    nc.vector.tensor_copy(
                        kts[i][:, n * 512 : (n + 1) * 512], ps[64 * i : 64 * i + 64, :]
                    )

            # ---- V projection, natural layout + ones column: [128, 32, 65] ----
            v1s = [
                v1p.tile([128, NKT, 65], bf16, tag="v1", name=f"v1{s}_{i}")
                for i in range(npair)
            ]
            for i in range(npair):
                nc.vector.memset(v1s[i][:, :, 64], 1.0)
            for kt in range(NKT):
                ps = psp.tile([128, 128], f32, tag="proj", name=f"psv{s}_{kt}")
                for ct in range(NCT):
                    nc.tensor.matmul(
                        ps,
                        lhsT=xkv_sb[:, ct, kt * 128 : (kt + 1) * 128],
                        rhs=wv_sb[:, s, ct, :],
                        start=(ct == 0),
                        stop=(ct == NCT - 1),
                    )
                for i in range(npair):
                    nc.vector.tensor_copy(
                        v1s[i][:, kt, 0:64], ps[:, 64 * i : 64 * i + 64]
                    )

            # ---- Q^T projection: [64, NQ] per block ----
            qts = [
                qtp.tile([64, NQ], bf16, tag="qt", name=f"qt{s}_{i}")
                for i in range(npair)
            ]
            for n in range(NQ // 512):
                ps = psp.tile([128, 512], f32, tag="proj", name=f"psq{s}_{n}")
                for ct in range(NCT):
                    nc.tensor.matmul(
                        ps,
                        lhsT=wq_sb[:, s, ct, :],
                        rhs=xq_sb[:, ct, n * 512 : (n + 1) * 512],
                        start=(ct == 0),
                        stop=(ct == NCT - 1),
                    )
                for i in range(npair):
                    nc.vector.tensor_copy(
                        qts[i][:, n * 512 : (n + 1) * 512], ps[64 * i : 64 * i + 64, :]
                    )

            # ---- attention per block ----
            for i, j in enumerate(js):
                po = pso.tile([65, NQ], f32, tag="pv", name=f"po{j}")
                for kt in range(NKT):
                    ss = pss.tile([128, NQ], f32, tag="ss", name=f"ss{j}_{kt}")
                    for qh in range(2):
                        nc.tensor.matmul(
                            ss[:, qh * 512 : (qh + 1) * 512],
                            lhsT=kts[i][:, kt * 128 : (kt + 1) * 128],
                            rhs=qts[i][:, qh * 512 : (qh + 1) * 512],
                            start=True,
                            stop=True,
                        )
                    pt = ptp.tile([128, NQ], bf16, tag="pt", name=f"pt{j}_{kt}")
                    nc.scalar.activation(pt, ss, EXP, scale=SCALE)
                    for qh in range(2):
                        nc.tensor.matmul(
                            po[:, qh * 512 : (qh + 1) * 512],
                            lhsT=v1s[i][:, kt, :],
                            rhs=pt[:, qh * 512 : (qh + 1) * 512],
                            start=(kt == 0),
                            stop=(kt == NKT - 1),
                            skip_group_check=True,
                        )
                # normalize: rows 0:64 multiplied by 1/rowsum (row 64)
                recip = sp.tile([1, NQ], f32, tag="recip", name=f"recip{j}")
                nc.vector.reciprocal(recip, po[64:65, :])
                recip_b = sp.tile([64, NQ], f32, tag="recipb", name=f"recipb{j}")
                nc.gpsimd.partition_broadcast(recip_b, recip)
                nc.vector.tensor_mul(
                    ot_dup[0:64, j * NQ : (j + 1) * NQ], po[0:64, :], recip_b
                )
                # shifted copy for rows 64:128 (ot_dup[64+d, c] = ot_dup[d, c+1])
                lo = j * NQ
                if j == 0:
                    nc.sync.dma_start(
                        out=ot_dup[64:128, 0 : NQ - 1], in_=ot_dup[0:64, 1:NQ]
                    )
                else:
                    nc.sync.dma_start(
                        out=ot_dup[64:128, lo - 1 : lo + NQ - 1],
                        in_=ot_dup[0:64, lo : lo + NQ],
                    )

        # ---- output projection: out[128r + j, :] = sum_u Y_u @ Wp_u + bp ----
        ot3 = ot_dup.rearrange("p (j u) -> p j u", u=12)
        for r in range(4):
            pout = pss.tile([128, C], f32, tag="ss", name=f"pout{r}")
            for t in range(NCT):
                for n0, n1 in ((0, 512), (512, C)):
                    nc.tensor.matmul(
                        pout[:, n0:n1],
                        lhsT=ot3[:, r * 128 : (r + 1) * 128, 2 * t],
                        rhs=wp_sb[:, t, n0:n1],
                        start=(t == 0),
                        stop=False,
                        skip_group_check=True,
                    )
            for n0, n1 in ((0, 512), (512, C)):
                nc.tensor.matmul(
                    pout[:, n0:n1],
                    lhsT=ones_row,
                    rhs=bp_sb[0:1, n0:n1],
                    start=False,
                    stop=True,
                    skip_group_check=True,
                )
            osb = outp.tile([128, C], f32, tag="osb", name=f"osb{r}")
            nc.vector.tensor_copy(osb, pout)
            nc.sync.dma_start(out=out[r * 128 : (r + 1) * 128, :], in_=osb)

    if not nc.is_finalized():
        nc.finalize()
    return nc


def make_in_maps(x_q, x_kv, Wq, Wkv, Wp, bp):
    bf = ml_dtypes.bfloat16
    xq_tt = np.ascontiguousarray(
        np.transpose(np.asarray(x_q, np.float32), (0, 2, 1))
    ).astype(bf)  # [B, C, NQ]
    xkv_tt = np.ascontiguousarray(
        np.transpose(np.asarray(x_kv, np.float32), (0, 2, 1))
    ).astype(bf)  # [B, C, NK]
    wq_b = np.asarray(Wq, np.float32).astype(bf)
    wkv_b = np.asarray(Wkv, np.float32).astype(bf)
    wp_b = np.ascontiguousarray(np.asarray(Wp, np.float32).astype(bf))
    bp_b = np.asarray(bp, np.float32).astype(bf).reshape(1, C)

    in_maps = []
    for i in range(8):
        slot_b = [(2 * i + k) % 4 for k in range(4)]
        wqp = np.zeros((4, C, 128), bf)
        wkp = np.zeros((4, C, 128), bf)
        wvp = np.zeros((4, C, 128), bf)
        for s, js in enumerate(SLOT_BLOCKS):
            for idx, j in enumerate(js):
                h = (6 * i + j) // 4
                wqp[s, :, idx * 64 : (idx + 1) * 64] = wq_b[:, h * 64 : (h + 1) * 64]
                wkp[s, :, idx * 64 : (idx + 1) * 64] = wkv_b[:, h * 64 : (h + 1) * 64]
                wvp[s, :, idx * 64 : (idx + 1) * 64] = wkv_b[
                    :, C + h * 64 : C + (h + 1) * 64
                ]
        in_maps.append(
            dict(
                xq_t=np.ascontiguousarray(xq_tt[slot_b]),
                xkv_t=np.ascontiguousarray(xkv_tt[slot_b]),
                wqp=wqp,
                wkp=wkp,
                wvp=wvp,
                wp=wp_b,
                bpv=bp_b,
            )
        )
    return in_maps


def run(inputs, trace=False, **kwargs):
    if "nc" not in _CACHE:
        _CACHE["nc"] = build_nc()
    nc = _CACHE["nc"]
    in_maps = make_in_maps(**inputs)
    res = run_bass_kernel_spmd(nc, in_maps, list(range(8)), trace=trace, **kwargs)
    out = np.concatenate([res.results[i]["out"] for i in range(8)], axis=0)
    return out.reshape(B, NQ, C).astype(np.float32), res


def kernel(x_q, x_kv, Wq, Wkv, Wp, bp):
    out, _ = run(dict(x_q=x_q, x_kv=x_kv, Wq=Wq, Wkv=Wkv, Wp=Wp, bp=bp))
    return out


# revision 9
# speedup vs baseline: 1.0719x; 1.0719x over previous
"""Cross-attention kernel for 8 TRN2 NeuronCores.

Problem: B=4, Nq=1024, Nkv=4096, C=768, H=12 heads, hd=64.

Sharding: by global (head, batch) block index g = h*4 + b.  Core i owns the 6
blocks g in [6i, 6i+6).  The reference's "scrambling" output reshape
([1,B,H,Nq,hd] -> transpose -> [B*Nq, C]) makes row r of the final projection
input consist of chunks 12r..12r+11 of the flat (h,b,q)-ordered attention
output -- so core i's 6 blocks are exactly the data needed for output rows
[512i, 512(i+1)): zero cross-core communication.

Per core, per block: Q/K/V projections in bf16 (Q^T and K^T in [hd, n]
layout, V natural with a fused ones-column for softmax row sums), scores
S^T = K^T.T @ Q^T per 128-row k-tile, exp on ScalarE (scale=hd^-0.5 folded
in, no max subtraction -- logits are ~N(0,1) so exp is safe), PV accumulated
over k-tiles into PSUM producing O^T [65, 1024] (row 64 = softmax denom).
Normalized O^T goes into OT_dup [128, 6144] (rows 64:128 = copy shifted left
by one column) so the final projection contracts over chunk-pairs (u,u+1)
with full K=128 matmuls reading stride-12 column APs.
"""

import os
import sys
from contextlib import ExitStack

import numpy as np
import ml_dtypes

for _p in ("/opt/trn_rl_repo", "/root/.axon_site/_ro/trn_rl_repo"):
    if _p not in sys.path and os.path.isdir(_p):
        sys.path.append(_p)

import concourse.bass as bass
import concourse.mybir as mybir
import concourse.tile as tile
from concourse import bacc
from concourse.bass_utils import run_bass_kernel_spmd

B, NQ, NK, C, H, HD = 4, 1024, 4096, 768, 12, 64
NCT = C // 128          # 6 contraction tiles of 128
NKT = NK // 128         # 32 k-tiles
NB = 6                  # blocks per core
SLOT_BLOCKS = [[0, 4], [1, 5], [2], [3]]  # local block ids per b-slot
SCALE = HD ** -0.5

f32 = mybir.dt.float32
bf16 = mybir.dt.bfloat16
EXP = mybir.ActivationFunctionType.Exp

_CACHE = {}


def build_nc():
    nc = bacc.Bacc()
    xq_t = nc.declare_dram_parameter("xq_t", [4, C, NQ], bf16, isOutput=False)
    xkv_t = nc.declare_dram_parameter("xkv_t", [4, C, NK], bf16, isOutput=False)
    wqp = nc.declare_dram_parameter("wqp", [4, C, 128], bf16, isOutput=False)
    wkp = nc.declare_dram_parameter("wkp", [4, C, 128], bf16, isOutput=False)
    wvp = nc.declare_dram_parameter("wvp", [4, C, 128], bf16, isOutput=False)
    wp = nc.declare_dram_parameter("wp", [C, C], bf16, isOutput=False)
    bpv = nc.declare_dram_parameter("bpv", [1, C], bf16, isOutput=False)
    out = nc.declare_dram_parameter("out", [512, C], f32, isOutput=True)

    with tile.TileContext(nc) as tc, ExitStack() as ctx:
        consts = ctx.enter_context(tc.tile_pool(name="consts", bufs=1))
        wpool = ctx.enter_context(tc.tile_pool(name="wpool", bufs=1))
        xkvp = ctx.enter_context(tc.tile_pool(name="xkvp", bufs=2))
        xqp = ctx.enter_context(tc.tile_pool(name="xqp", bufs=1))
        ktp = ctx.enter_context(tc.tile_pool(name="ktp", bufs=2))
        v1p = ctx.enter_context(tc.tile_pool(name="v1p", bufs=2))
        qtp = ctx.enter_context(tc.tile_pool(name="qtp", bufs=2))
        ptp = ctx.enter_context(tc.tile_pool(name="ptp", bufs=4))
        otp = ctx.enter_context(tc.tile_pool(name="otp", bufs=1))
        sp = ctx.enter_context(tc.tile_pool(name="sp", bufs=2))
        outp = ctx.enter_context(tc.tile_pool(name="outp", bufs=2))
        psp = ctx.enter_context(tc.tile_pool(name="psp", bufs=2, space="PSUM"))
        pss = ctx.enter_context(tc.tile_pool(name="pss", bufs=2, space="PSUM"))
        pso = ctx.enter_context(tc.tile_pool(name="pso", bufs=1, space="PSUM"))

        # ---- weights / constants (loaded once) ----
        wp_sb = wpool.tile([128, NCT, C], bf16, name="wp_sb")
        nc.sync.dma_start(out=wp_sb, in_=wp.rearrange("(t p) n -> p t n", p=128))
        wq_sb = wpool.tile([128, 4, NCT, 128], bf16, name="wq_sb")
        nc.sync.dma_start(out=wq_sb, in_=wqp.rearrange("s (t p) m -> p s t m", p=128))
        wk_sb = wpool.tile([128, 4, NCT, 128], bf16, name="wk_sb")
        nc.sync.dma_start(out=wk_sb, in_=wkp.rearrange("s (t p) m -> p s t m", p=128))
        wv_sb = wpool.tile([128, 4, NCT, 128], bf16, name="wv_sb")
        nc.sync.dma_start(out=wv_sb, in_=wvp.rearrange("s (t p) m -> p s t m", p=128))
        bp_sb = consts.tile([1, C], bf16, name="bp_sb")
        nc.sync.dma_start(out=bp_sb, in_=bpv[:])
        ones_row = consts.tile([1, 128], bf16, name="ones_row")
        nc.vector.memset(ones_row, 1.0)
        # attention output, chunk-major: col 12*j_row + u;  rows 64:128 shifted
        ot_dup = otp.tile([128, NB * NQ], bf16, name="ot_dup")

        for s in range(4):
            js = SLOT_BLOCKS[s]
            npair = len(js)
            xkv_sb = xkvp.tile([128, NCT, NK], bf16, tag="xkv", name=f"xkv{s}")
            src = xkv_t[s].rearrange("(t p) n -> p t n", p=128)
            for ct in range(NCT):
                for ch in range(4):  # chunked so consumers start early
                    nc.sync.dma_start(
                        out=xkv_sb[:, ct, ch * 1024 : (ch + 1) * 1024],
                        in_=src[:, ct, ch * 1024 : (ch + 1) * 1024],
                    )
            xq_sb = xqp.tile([128, NCT, NQ], bf16, tag="xq", name=f"xq{s}")
            nc.sync.dma_start(
                out=xq_sb, in_=xq_t[s].rearrange("(t p) n -> p t n", p=128)
            )

            # ---- Q^T projection (first: small DMA, warms PE early) ----
            qts = [
                qtp.tile([64, NQ], bf16, tag="qt", name=f"qt{s}_{i}")
                for i in range(npair)
            ]
            for n in range(NQ // 512):
                ps = psp.tile([128, 512], f32, tag="proj", name=f"psq{s}_{n}")
                for ct in range(NCT):
                    nc.tensor.matmul(
                        ps,
                        lhsT=wq_sb[:, s, ct, :],
                        rhs=xq_sb[:, ct, n * 512 : (n + 1) * 512],
                        start=(ct == 0),
                        stop=(ct == NCT - 1),
                    )
                for i in range(npair):
                    nc.vector.tensor_copy(
                        qts[i][:, n * 512 : (n + 1) * 512], ps[64 * i : 64 * i + 64, :]
                    )

            # ---- K^T projection: [64, NK] per block (pair-packed M=128) ----
            kts = [
                ktp.tile([64, NK], bf16, tag="kt", name=f"kt{s}_{i}")
                for i in range(npair)
            ]
            for n in range(NK // 512):
                ps = psp.tile([128, 512], f32, tag="proj", name=f"psk{s}_{n}")
                for ct in range(NCT):
                    nc.tensor.matmul(
                        ps,
                        lhsT=wk_sb[:, s, ct, :],
                        rhs=xkv_sb[:, ct, n * 512 : (n + 1) * 512],
                        start=(ct == 0),
                        stop=(ct == NCT - 1),
                    )
                for i in range(npair):
                    nc.vector.tensor_copy(
                        kts[i][:, n * 512 : (n + 1) * 512], ps[64 * i : 64 * i + 64, :]
                    )

            # ---- V projection, natural layout + ones column: [128, 32, 65] ----
            v1s = [
                v1p.tile([128, NKT, 65], bf16, tag="v1", name=f"v1{s}_{i}")
                for i in range(npair)
            ]
            for i in range(npair):
                nc.vector.memset(v1s[i][:, :, 64], 1.0)
            for kt in range(NKT):
                ps = psp.tile([128, 128], f32, tag="proj", name=f"psv{s}_{kt}")
                for ct in range(NCT):
                    nc.tensor.matmul(
                        ps,
                        lhsT=xkv_sb[:, ct, kt * 128 : (kt + 1) * 128],
                        rhs=wv_sb[:, s, ct, :],
                        start=(ct == 0),
                        stop=(ct == NCT - 1),
                    )
                for i in range(npair):
                    nc.vector.tensor_copy(
                        v1s[i][:, kt, 0:64], ps[:, 64 * i : 64 * i + 64]
                    )

            # ---- attention per block ----
            for i, j in enumerate(js):
                po = pso.tile([65, NQ], f32, tag="pv", name=f"po{j}")
                for kt in range(NKT):
                    ss = pss.tile([128, NQ], f32, tag="ss", name=f"ss{j}_{kt}")
                    for qh in range(2):
                        nc.tensor.matmul(
                            ss[:, qh * 512 : (qh + 1) * 512],
                            lhsT=kts[i][:, kt * 128 : (kt + 1) * 128],
                            rhs=qts[i][:, qh * 512 : (qh + 1) * 512],
                            start=True,
                            stop=True,
                        )
                    pt = ptp.tile([128, NQ], bf16, tag="pt", name=f"pt{j}_{kt}")
                    nc.scalar.activation(pt, ss, EXP, scale=SCALE)
                    for qh in range(2):
                        nc.tensor.matmul(
                            po[:, qh * 512 : (qh + 1) * 512],
                            lhsT=v1s[i][:, kt, :],
                            rhs=pt[:, qh * 512 : (qh + 1) * 512],
                            start=(kt == 0),
                            stop=(kt == NKT - 1),
                            skip_group_check=True,
                        )
                # normalize: rows 0:64 multiplied by 1/rowsum (row 64)
                recip = sp.tile([1, NQ], f32, tag="recip", name=f"recip{j}")
                nc.vector.reciprocal(recip, po[64:65, :])
                recip_b = sp.tile([64, NQ], f32, tag="recipb", name=f"recipb{j}")
                nc.gpsimd.partition_broadcast(recip_b, recip)
                nc.vector.tensor_mul(
                    ot_dup[0:64, j * NQ : (j + 1) * NQ], po[0:64, :], recip_b
                )
                # shifted copy for rows 64:128 (ot_dup[64+d, c] = ot_dup[d, c+1])
                lo = j * NQ
                if j == 0:
                    nc.sync.dma_start(
                        out=ot_dup[64:128, 0 : NQ - 1], in_=ot_dup[0:64, 1:NQ]
                    )
                else:
                    nc.sync.dma_start(
                        out=ot_dup[64:128, lo - 1 : lo + NQ - 1],
                        in_=ot_dup[0:64, lo : lo + NQ],
                    )

        # ---- output projection: out[128r + j, :] = sum_u Y_u @ Wp_u + bp ----
        ot3 = ot_dup.rearrange("p (j u) -> p j u", u=12)
        for r in range(4):
            osb = outp.tile([128, C], f32, tag="osb", name=f"osb{r}")
            for n0, n1 in ((0, 384), (384, C)):
                pout = psp.tile([128, 384], f32, tag="proj", name=f"pout{r}_{n0}")
                for t in range(NCT):
                    nc.tensor.matmul(
                        pout,
                        lhsT=ot3[:, r * 128 : (r + 1) * 128, 2 * t],
                        rhs=wp_sb[:, t, n0:n1],
                        start=(t == 0),
                        stop=False,
                        skip_group_check=True,
                    )
                nc.tensor.matmul(
                    pout,
                    lhsT=ones_row,
                    rhs=bp_sb[0:1, n0:n1],
                    start=False,
                    stop=True,
                    skip_group_check=True,
                )
                nc.vector.tensor_copy(osb[:, n0:n1], pout)
            nc.sync.dma_start(out=out[r * 128 : (r + 1) * 128, :], in_=osb)

    if not nc.is_finalized():
        nc.finalize()
    return nc


def make_in_maps(x_q, x_kv, Wq, Wkv, Wp, bp):
    bf = ml_dtypes.bfloat16
    xq_tt = np.ascontiguousarray(
        np.transpose(np.asarray(x_q, np.float32), (0, 2, 1))
    ).astype(bf)  # [B, C, NQ]
    xkv_tt = np.ascontiguousarray(
        np.transpose(np.asarray(x_kv, np.float32), (0, 2, 1))
    ).astype(bf)  # [B, C, NK]
    wq_b = np.asarray(Wq, np.float32).astype(bf)
    wkv_b = np.asarray(Wkv, np.float32).astype(bf)
    wp_b = np.ascontiguousarray(np.asarray(Wp, np.float32).astype(bf))
    bp_b = np.asarray(bp, np.float32).astype(bf).reshape(1, C)

    in_maps = []
    for i in range(8):
        slot_b = [(2 * i + k) % 4 for k in range(4)]
        wqp = np.zeros((4, C, 128), bf)
        wkp = np.zeros((4, C, 128), bf)
        wvp = np.zeros((4, C, 128), bf)
        for s, js in enumerate(SLOT_BLOCKS):
            for idx, j in enumerate(js):
                h = (6 * i + j) // 4
                wqp[s, :, idx * 64 : (idx + 1) * 64] = wq_b[:, h * 64 : (h + 1) * 64]
                wkp[s, :, idx * 64 : (idx + 1) * 64] = wkv_b[:, h * 64 : (h + 1) * 64]
                wvp[s, :, idx * 64 : (idx + 1) * 64] = wkv_b[
                    :, C + h * 64 : C + (h + 1) * 64
                ]
        in_maps.append(
            dict(
                xq_t=np.ascontiguousarray(xq_tt[slot_b]),
                xkv_t=np.ascontiguousarray(xkv_tt[slot_b]),
                wqp=wqp,
                wkp=wkp,
                wvp=wvp,
                wp=wp_b,
                bpv=bp_b,
            )
        )
    return in_maps


def run(inputs, trace=False, **kwargs):
    if "nc" not in _CACHE:
        _CACHE["nc"] = build_nc()
    nc = _CACHE["nc"]
    in_maps = make_in_maps(**inputs)
    res = run_bass_kernel_spmd(nc, in_maps, list(range(8)), trace=trace, **kwargs)
    out = np.concatenate([res.results[i]["out"] for i in range(8)], axis=0)
    return out.reshape(B, NQ, C).astype(np.float32), res


def kernel(x_q, x_kv, Wq, Wkv, Wp, bp):
    out, _ = run(dict(x_q=x_q, x_kv=x_kv, Wq=Wq, Wkv=Wkv, Wp=Wp, bp=bp))
    return out


# revision 19
# speedup vs baseline: 1.1969x; 1.1166x over previous
"""Cross-attention kernel for 8 TRN2 NeuronCores.

Problem: B=4, Nq=1024, Nkv=4096, C=768, H=12 heads, hd=64.

Sharding: by global (head, batch) block index g = h*4 + b.  Core i owns the 6
blocks g in [6i, 6i+6).  The reference's "scrambling" output reshape
([1,B,H,Nq,hd] -> transpose -> [B*Nq, C]) makes row r of the final projection
input consist of chunks 12r..12r+11 of the flat (h,b,q)-ordered attention
output -- so core i's 6 blocks are exactly the data needed for output rows
[512i, 512(i+1)): zero cross-core communication.

Per core, per block: Q/K/V projections in bf16 (Q^T and K^T in [hd, n]
layout, V natural with a fused ones-column for softmax row sums), scores
S^T = K^T.T @ Q^T per 128-row k-tile, exp on ScalarE (scale=hd^-0.5 folded
in, no max subtraction -- logits are ~N(0,1) so exp is safe), PV accumulated
over k-tiles into PSUM producing O^T [65, 1024] (row 64 = softmax denom).
Normalized O^T goes into OT_dup [128, 6144] (rows 64:128 = copy shifted left
by one column) so the final projection contracts over chunk-pairs (u,u+1)
with full K=128 matmuls reading stride-12 column APs.
"""

import os
import sys
from contextlib import ExitStack

import numpy as np
import ml_dtypes

for _p in ("/opt/trn_rl_repo", "/root/.axon_site/_ro/trn_rl_repo"):
    if _p not in sys.path and os.path.isdir(_p):
        sys.path.append(_p)

import concourse.bass as bass
import concourse.mybir as mybir
import concourse.tile as tile
from concourse import bacc
from concourse.bass_utils import run_bass_kernel_spmd

B, NQ, NK, C, H, HD = 4, 1024, 4096, 768, 12, 64
NCT = C // 128          # 6 contraction tiles of 128
NKT = NK // 128         # 32 k-tiles
NB = 6                  # blocks per core
SLOT_BLOCKS = [[0, 4], [1, 5], [2], [3]]  # local block ids per b-slot
SCALE = HD ** -0.5

f32 = mybir.dt.float32
bf16 = mybir.dt.bfloat16
EXP = mybir.ActivationFunctionType.Exp

_CACHE = {}


def build_nc():
    nc = bacc.Bacc()
    xq_t = nc.declare_dram_parameter("xq_t", [4, C, NQ], bf16, isOutput=False)
    xkv_t = nc.declare_dram_parameter("xkv_t", [4, C, NK], bf16, isOutput=False)
    wqp = nc.declare_dram_parameter("wqp", [4, C, 128], bf16, isOutput=False)
    wkp = nc.declare_dram_parameter("wkp", [4, C, 128], bf16, isOutput=False)
    wvp = nc.declare_dram_parameter("wvp", [4, C, 128], bf16, isOutput=False)
    wp = nc.declare_dram_parameter("wp", [C, C], bf16, isOutput=False)
    bpv = nc.declare_dram_parameter("bpv", [1, C], bf16, isOutput=False)
    out = nc.declare_dram_parameter("out", [512, C], f32, isOutput=True)

    with tile.TileContext(nc) as tc, ExitStack() as ctx:
        consts = ctx.enter_context(tc.tile_pool(name="consts", bufs=1))
        wpool = ctx.enter_context(tc.tile_pool(name="wpool", bufs=1))
        xkvp = ctx.enter_context(tc.tile_pool(name="xkvp", bufs=2))
        xqp = ctx.enter_context(tc.tile_pool(name="xqp", bufs=1))
        ktp = ctx.enter_context(tc.tile_pool(name="ktp", bufs=2))
        v1p = ctx.enter_context(tc.tile_pool(name="v1p", bufs=2))
        qtp = ctx.enter_context(tc.tile_pool(name="qtp", bufs=2))
        ptp = ctx.enter_context(tc.tile_pool(name="ptp", bufs=4))
        otp = ctx.enter_context(tc.tile_pool(name="otp", bufs=1))
        sp = ctx.enter_context(tc.tile_pool(name="sp", bufs=1))
        outp = ctx.enter_context(tc.tile_pool(name="outp", bufs=2))
        psp = ctx.enter_context(tc.tile_pool(name="psp", bufs=2, space="PSUM"))
        pss = ctx.enter_context(tc.tile_pool(name="pss", bufs=2, space="PSUM"))
        pso = ctx.enter_context(tc.tile_pool(name="pso", bufs=1, space="PSUM"))

        # ---- weights / constants (loaded once) ----
        wp_sb = wpool.tile([128, NCT, C], bf16, name="wp_sb")
        nc.sync.dma_start(out=wp_sb, in_=wp.rearrange("(t p) n -> p t n", p=128))
        wq_sb = wpool.tile([128, 4, NCT, 128], bf16, name="wq_sb")
        nc.sync.dma_start(
            out=wq_sb, in_=wqp.rearrange("s (t p) m -> p s t m", p=128)
        )
        wk_sb = wpool.tile([128, 4, NCT, 128], bf16, name="wk_sb")
        nc.sync.dma_start(
            out=wk_sb, in_=wkp.rearrange("s (t p) m -> p s t m", p=128)
        )
        wv_sb = wpool.tile([128, 4, NCT, 128], bf16, name="wv_sb")
        nc.sync.dma_start(
            out=wv_sb, in_=wvp.rearrange("s (t p) m -> p s t m", p=128)
        )
        bp_sb = consts.tile([1, C], bf16, name="bp_sb")
        nc.sync.dma_start(out=bp_sb, in_=bpv[:])
        ones_row = consts.tile([1, 128], bf16, name="ones_row")
        nc.vector.memset(ones_row, 1.0)
        # attention output, chunk-major: col 12*j_row + u;  rows 64:128 shifted
        ot_dup = otp.tile([128, NB * NQ], bf16, name="ot_dup")

        for s in range(4):
            js = SLOT_BLOCKS[s]
            npair = len(js)
            xkv_sb = xkvp.tile([128, NCT, NK], bf16, tag="xkv", name=f"xkv{s}")
            src = xkv_t[s].rearrange("(t p) n -> p t n", p=128)
            for ct in range(NCT):
                nc.sync.dma_start(out=xkv_sb[:, ct, :], in_=src[:, ct, :])
            xq_sb = xqp.tile([128, NCT, NQ], bf16, tag="xq", name=f"xq{s}")
            nc.sync.dma_start(
                out=xq_sb, in_=xq_t[s].rearrange("(t p) n -> p t n", p=128)
            )

            # ---- Q^T projection (first: small DMA, warms PE early) ----
            qts = [
                qtp.tile([64, NQ], bf16, tag="qt", name=f"qt{s}_{i}")
                for i in range(npair)
            ]
            for n in range(NQ // 512):
                ps = psp.tile([128, 512], f32, tag="proj", name=f"psq{s}_{n}")
                for ct in range(NCT):
                    nc.tensor.matmul(
                        ps,
                        lhsT=wq_sb[:, s, ct, :],
                        rhs=xq_sb[:, ct, n * 512 : (n + 1) * 512],
                        start=(ct == 0),
                        stop=(ct == NCT - 1),
                    )
                for i in range(npair):
                    nc.vector.tensor_copy(
                        qts[i][:, n * 512 : (n + 1) * 512], ps[64 * i : 64 * i + 64, :]
                    )

            # ---- K^T projection: [64, NK] per block (pair-packed M=128) ----
            kts = [
                ktp.tile([64, NK], bf16, tag="kt", name=f"kt{s}_{i}")
                for i in range(npair)
            ]
            for n in range(NK // 512):
                ps = psp.tile([128, 512], f32, tag="proj", name=f"psk{s}_{n}")
                for ct in range(NCT):
                    nc.tensor.matmul(
                        ps,
                        lhsT=wk_sb[:, s, ct, :],
                        rhs=xkv_sb[:, ct, n * 512 : (n + 1) * 512],
                        start=(ct == 0),
                        stop=(ct == NCT - 1),
                    )
                for i in range(npair):
                    nc.vector.tensor_copy(
                        kts[i][:, n * 512 : (n + 1) * 512], ps[64 * i : 64 * i + 64, :]
                    )

            # ---- V projection, natural layout + ones column: [128, 32, 65] ----
            v1s = [
                v1p.tile([128, NKT, 65], bf16, tag="v1", name=f"v1{s}_{i}")
                for i in range(npair)
            ]
            for i in range(npair):
                nc.vector.memset(v1s[i][:, :, 64], 1.0)
            for kt in range(NKT):
                ps = psp.tile([128, 128], f32, tag="proj", name=f"psv{s}_{kt}")
                for ct in range(NCT):
                    nc.tensor.matmul(
                        ps,
                        lhsT=xkv_sb[:, ct, kt * 128 : (kt + 1) * 128],
                        rhs=wv_sb[:, s, ct, :],
                        start=(ct == 0),
                        stop=(ct == NCT - 1),
                    )
                for i in range(npair):
                    nc.vector.tensor_copy(
                        v1s[i][:, kt, 0:64], ps[:, 64 * i : 64 * i + 64]
                    )

            # ---- attention per block ----
            for i, j in enumerate(js):
                po = pso.tile([65, NQ], f32, tag="pv", name=f"po{j}")
                for kt in range(NKT):
                    ss = pss.tile([128, NQ], f32, tag="ss", name=f"ss{j}_{kt}")
                    for qh in range(2):
                        nc.tensor.matmul(
                            ss[:, qh * 512 : (qh + 1) * 512],
                            lhsT=kts[i][:, kt * 128 : (kt + 1) * 128],
                            rhs=qts[i][:, qh * 512 : (qh + 1) * 512],
                            start=True,
                            stop=True,
                        )
                    pt = ptp.tile([128, NQ], bf16, tag="pt", name=f"pt{j}_{kt}")
                    nc.scalar.activation(pt, ss, EXP, scale=SCALE)
                    for qh in range(2):
                        nc.tensor.matmul(
                            po[:, qh * 512 : (qh + 1) * 512],
                            lhsT=v1s[i][:, kt, :],
                            rhs=pt[:, qh * 512 : (qh + 1) * 512],
                            start=(kt == 0),
                            stop=(kt == NKT - 1),
                            skip_group_check=True,
                        )
                # normalize: rows 0:64 multiplied by 1/rowsum (row 64)
                zrow = sp.tile([1, NQ], f32, tag="zrow", name=f"zrow{j}")
                nc.vector.tensor_copy(zrow, po[64:65, :])
                recip = sp.tile([1, NQ], f32, tag="recip", name=f"recip{j}")
                nc.vector.reciprocal_approx_fast(out=recip, in_=zrow)
                recip_b = sp.tile([64, NQ], f32, tag="recipb", name=f"recipb{j}")
                nc.gpsimd.partition_broadcast(recip_b, recip)
                nc.vector.tensor_mul(
                    ot_dup[0:64, j * NQ : (j + 1) * NQ], po[0:64, :], recip_b
                )
                # shifted copy for rows 64:128 (ot_dup[64+d, c] = ot_dup[d, c+1])
                lo = j * NQ
                if j == 0:
                    nc.gpsimd.dma_start(
                        out=ot_dup[64:128, 0 : NQ - 1], in_=ot_dup[0:64, 1:NQ]
                    )
                else:
                    nc.gpsimd.dma_start(
                        out=ot_dup[64:128, lo - 1 : lo + NQ - 1],
                        in_=ot_dup[0:64, lo : lo + NQ],
                    )

        # ---- output projection: out[128r + j, :] = sum_u Y_u @ Wp_u + bp ----
        ot3 = ot_dup.rearrange("p (j u) -> p j u", u=12)
        for r in range(4):
            osb = outp.tile([128, C], f32, tag="osb", name=f"osb{r}")
            for n0, n1 in ((0, 384), (384, C)):
                pout = psp.tile([128, 384], f32, tag="proj", name=f"pout{r}_{n0}")
                for t in range(NCT):
                    nc.tensor.matmul(
                        pout,
                        lhsT=ot3[:, r * 128 : (r + 1) * 128, 2 * t],
                        rhs=wp_sb[:, t, n0:n1],
                        start=(t == 0),
                        stop=False,
                        skip_group_check=True,
                    )
                nc.tensor.matmul(
                    pout,
                    lhsT=ones_row,
                    rhs=bp_sb[0:1, n0:n1],
                    start=False,
                    stop=True,
                    skip_group_check=True,
                )
                nc.vector.tensor_copy(osb[:, n0:n1], pout)
            nc.gpsimd.dma_start(out=out[r * 128 : (r + 1) * 128, :], in_=osb)

    if not nc.is_finalized():
        nc.finalize()
    return nc


def make_in_maps(x_q, x_kv, Wq, Wkv, Wp, bp):
    bf = ml_dtypes.bfloat16
    xq_tt = np.ascontiguousarray(
        np.transpose(np.asarray(x_q, np.float32), (0, 2, 1))
    ).astype(bf)  # [B, C, NQ]
    xkv_tt = np.ascontiguousarray(
        np.transpose(np.asarray(x_kv, np.float32), (0, 2, 1))
    ).astype(bf)  # [B, C, NK]
    wq_b = np.asarray(Wq, np.float32).astype(bf)
    wkv_b = np.asarray(Wkv, np.float32).astype(bf)
    wp_b = np.ascontiguousarray(np.asarray(Wp, np.float32).astype(bf))
    bp_b = np.asarray(bp, np.float32).astype(bf).reshape(1, C)

    in_maps = []
    for i in range(8):
        slot_b = [(2 * i + k) % 4 for k in range(4)]
        wqp = np.zeros((4, C, 128), bf)
        wkp = np.zeros((4, C, 128), bf)
        wvp = np.zeros((4, C, 128), bf)
        for s, js in enumerate(SLOT_BLOCKS):
            for idx, j in enumerate(js):
                h = (6 * i + j) // 4
                wqp[s, :, idx * 64 : (idx + 1) * 64] = wq_b[:, h * 64 : (h + 1) * 64]
                wkp[s, :, idx * 64 : (idx + 1) * 64] = wkv_b[:, h * 64 : (h + 1) * 64]
                wvp[s, :, idx * 64 : (idx + 1) * 64] = wkv_b[
                    :, C + h * 64 : C + (h + 1) * 64
                ]
        in_maps.append(
            dict(
                xq_t=np.ascontiguousarray(xq_tt[slot_b]),
                xkv_t=np.ascontiguousarray(xkv_tt[slot_b]),
                wqp=wqp,
                wkp=wkp,
                wvp=wvp,
                wp=wp_b,
                bpv=bp_b,
            )
        )
    return in_maps


def run(inputs, trace=False, **kwargs):
    if "nc" not in _CACHE:
        _CACHE["nc"] = build_nc()
    nc = _CACHE["nc"]
    in_maps = make_in_maps(**inputs)
    res = run_bass_kernel_spmd(nc, in_maps, list(range(8)), trace=trace, **kwargs)
    out = np.concatenate([res.results[i]["out"] for i in range(8)], axis=0)
    return out.reshape(B, NQ, C).astype(np.float32), res


def kernel(x_q, x_kv, Wq, Wkv, Wp, bp):
    out, _ = run(dict(x_q=x_q, x_kv=x_kv, Wq=Wq, Wkv=Wkv, Wp=Wp, bp=bp))
    return out


# revision 20
# speedup vs baseline: 1.2247x; 1.0232x over previous
"""Cross-attention kernel for 8 TRN2 NeuronCores.

Problem: B=4, Nq=1024, Nkv=4096, C=768, H=12 heads, hd=64.

Sharding: by global (head, batch) block index g = h*4 + b.  Core i owns the 6
blocks g in [6i, 6i+6).  The reference's "scrambling" output reshape
([1,B,H,Nq,hd] -> transpose -> [B*Nq, C]) makes row r of the final projection
input consist of chunks 12r..12r+11 of the flat (h,b,q)-ordered attention
output -- so core i's 6 blocks are exactly the data needed for output rows
[512i, 512(i+1)): zero cross-core communication.

Per core, per block: Q/K/V projections in bf16 (Q^T and K^T in [hd, n]
layout, V natural with a fused ones-column for softmax row sums), scores
S^T = K^T.T @ Q^T per 128-row k-tile, exp on ScalarE (scale=hd^-0.5 folded
in, no max subtraction -- logits are ~N(0,1) so exp is safe), PV accumulated
over k-tiles into PSUM producing O^T [65, 1024] (row 64 = softmax denom).
Normalized O^T goes into OT_dup [128, 6144] (rows 64:128 = copy shifted left
by one column) so the final projection contracts over chunk-pairs (u,u+1)
with full K=128 matmuls reading stride-12 column APs.
"""

import os
import sys
from contextlib import ExitStack

import numpy as np
import ml_dtypes

for _p in ("/opt/trn_rl_repo", "/root/.axon_site/_ro/trn_rl_repo"):
    if _p not in sys.path and os.path.isdir(_p):
        sys.path.append(_p)

import concourse.bass as bass
import concourse.mybir as mybir
import concourse.tile as tile
from concourse import bacc
from concourse.bass_utils import run_bass_kernel_spmd

B, NQ, NK, C, H, HD = 4, 1024, 4096, 768, 12, 64
NCT = C // 128          # 6 contraction tiles of 128
NKT = NK // 128         # 32 k-tiles
NB = 6                  # blocks per core
SLOT_BLOCKS = [[0, 4], [1, 5], [2], [3]]  # local block ids per b-slot
SCALE = HD ** -0.5

f32 = mybir.dt.float32
bf16 = mybir.dt.bfloat16
EXP = mybir.ActivationFunctionType.Exp

_CACHE = {}


def build_nc():
    nc = bacc.Bacc()
    xq_t = nc.declare_dram_parameter("xq_t", [4, C, NQ], bf16, isOutput=False)
    xkv_t = nc.declare_dram_parameter("xkv_t", [4, C, NK], bf16, isOutput=False)
    wqp = nc.declare_dram_parameter("wqp", [4, C, 128], bf16, isOutput=False)
    wkp = nc.declare_dram_parameter("wkp", [4, C, 128], bf16, isOutput=False)
    wvp = nc.declare_dram_parameter("wvp", [4, C, 128], bf16, isOutput=False)
    wp = nc.declare_dram_parameter("wp", [C, C], bf16, isOutput=False)
    bpv = nc.declare_dram_parameter("bpv", [1, C], bf16, isOutput=False)
    out = nc.declare_dram_parameter("out", [512, C], f32, isOutput=True)

    with tile.TileContext(nc) as tc, ExitStack() as ctx:
        consts = ctx.enter_context(tc.tile_pool(name="consts", bufs=1))
        wpool = ctx.enter_context(tc.tile_pool(name="wpool", bufs=1))
        xkvp = ctx.enter_context(tc.tile_pool(name="xkvp", bufs=2))
        xqp = ctx.enter_context(tc.tile_pool(name="xqp", bufs=1))
        ktp = ctx.enter_context(tc.tile_pool(name="ktp", bufs=2))
        v1p = ctx.enter_context(tc.tile_pool(name="v1p", bufs=2))
        qtp = ctx.enter_context(tc.tile_pool(name="qtp", bufs=2))
        ptp = ctx.enter_context(tc.tile_pool(name="ptp", bufs=4))
        otp = ctx.enter_context(tc.tile_pool(name="otp", bufs=1))
        sp = ctx.enter_context(tc.tile_pool(name="sp", bufs=1))
        outp = ctx.enter_context(tc.tile_pool(name="outp", bufs=2))
        psp = ctx.enter_context(tc.tile_pool(name="psp", bufs=2, space="PSUM"))
        pss = ctx.enter_context(tc.tile_pool(name="pss", bufs=2, space="PSUM"))
        pso = ctx.enter_context(tc.tile_pool(name="pso", bufs=1, space="PSUM"))

        def load_slot(s):
            """xkv in column-halves so K-proj n-chunks 0-3 start at half-arrival."""
            xkv_sb = xkvp.tile([128, NCT, NK], bf16, tag="xkv", name=f"xkv{s}")
            src = xkv_t[s].rearrange("(t p) n -> p t n", p=128)
            for half in range(2):
                for ct in range(NCT):
                    nc.sync.dma_start(
                        out=xkv_sb[:, ct, half * 2048 : (half + 1) * 2048],
                        in_=src[:, ct, half * 2048 : (half + 1) * 2048],
                    )
            xq_sb = xqp.tile([128, NCT, NQ], bf16, tag="xq", name=f"xq{s}")
            nc.sync.dma_start(
                out=xq_sb, in_=xq_t[s].rearrange("(t p) n -> p t n", p=128)
            )
            return xkv_sb, xq_sb

        # slot-0 inputs issue first so the PE isn't starved behind weight DMAs
        slot_data = {0: load_slot(0)}

        # ---- weights / constants (loaded once) ----
        wq_sb = wpool.tile([128, 4, NCT, 128], bf16, name="wq_sb")
        nc.sync.dma_start(
            out=wq_sb, in_=wqp.rearrange("s (t p) m -> p s t m", p=128)
        )
        wk_sb = wpool.tile([128, 4, NCT, 128], bf16, name="wk_sb")
        nc.sync.dma_start(
            out=wk_sb, in_=wkp.rearrange("s (t p) m -> p s t m", p=128)
        )
        wv_sb = wpool.tile([128, 4, NCT, 128], bf16, name="wv_sb")
        nc.sync.dma_start(
            out=wv_sb, in_=wvp.rearrange("s (t p) m -> p s t m", p=128)
        )
        wp_sb = wpool.tile([128, NCT, C], bf16, name="wp_sb")
        nc.gpsimd.dma_start(out=wp_sb, in_=wp.rearrange("(t p) n -> p t n", p=128))
        bp_sb = consts.tile([1, C], bf16, name="bp_sb")
        nc.gpsimd.dma_start(out=bp_sb, in_=bpv[:])
        ones_row = consts.tile([1, 128], bf16, name="ones_row")
        nc.vector.memset(ones_row, 1.0)
        # attention output, chunk-major: col 12*j_row + u;  rows 64:128 shifted
        ot_dup = otp.tile([128, NB * NQ], bf16, name="ot_dup")

        for s in range(4):
            js = SLOT_BLOCKS[s]
            npair = len(js)
            xkv_sb, xq_sb = slot_data.pop(s)
            if s + 1 < 4:
                slot_data[s + 1] = load_slot(s + 1)

            # ---- Q^T projection (first: small DMA, warms PE early) ----
            qts = [
                qtp.tile([64, NQ], bf16, tag="qt", name=f"qt{s}_{i}")
                for i in range(npair)
            ]
            for n in range(NQ // 512):
                ps = psp.tile([128, 512], f32, tag="proj", name=f"psq{s}_{n}")
                for ct in range(NCT):
                    nc.tensor.matmul(
                        ps,
                        lhsT=wq_sb[:, s, ct, :],
                        rhs=xq_sb[:, ct, n * 512 : (n + 1) * 512],
                        start=(ct == 0),
                        stop=(ct == NCT - 1),
                    )
                for i in range(npair):
                    nc.vector.tensor_copy(
                        qts[i][:, n * 512 : (n + 1) * 512], ps[64 * i : 64 * i + 64, :]
                    )

            # ---- K^T projection: [64, NK] per block (pair-packed M=128) ----
            kts = [
                ktp.tile([64, NK], bf16, tag="kt", name=f"kt{s}_{i}")
                for i in range(npair)
            ]
            for n in range(NK // 512):
                ps = psp.tile([128, 512], f32, tag="proj", name=f"psk{s}_{n}")
                for ct in range(NCT):
                    nc.tensor.matmul(
                        ps,
                        lhsT=wk_sb[:, s, ct, :],
                        rhs=xkv_sb[:, ct, n * 512 : (n + 1) * 512],
                        start=(ct == 0),
                        stop=(ct == NCT - 1),
                    )
                for i in range(npair):
                    nc.vector.tensor_copy(
                        kts[i][:, n * 512 : (n + 1) * 512], ps[64 * i : 64 * i + 64, :]
                    )

            # ---- V projection, natural layout + ones column: [128, 32, 65] ----
            v1s = [
                v1p.tile([128, NKT, 65], bf16, tag="v1", name=f"v1{s}_{i}")
                for i in range(npair)
            ]
            for i in range(npair):
                nc.vector.memset(v1s[i][:, :, 64], 1.0)
            for kt in range(NKT):
                ps = psp.tile([128, 128], f32, tag="proj", name=f"psv{s}_{kt}")
                for ct in range(NCT):
                    nc.tensor.matmul(
                        ps,
                        lhsT=xkv_sb[:, ct, kt * 128 : (kt + 1) * 128],
                        rhs=wv_sb[:, s, ct, :],
                        start=(ct == 0),
                        stop=(ct == NCT - 1),
                    )
                for i in range(npair):
                    nc.vector.tensor_copy(
                        v1s[i][:, kt, 0:64], ps[:, 64 * i : 64 * i + 64]
                    )

            # ---- attention per block ----
            for i, j in enumerate(js):
                po = pso.tile([65, NQ], f32, tag="pv", name=f"po{j}")
                for kt in range(NKT):
                    ss = pss.tile([128, NQ], f32, tag="ss", name=f"ss{j}_{kt}")
                    for qh in range(2):
                        nc.tensor.matmul(
                            ss[:, qh * 512 : (qh + 1) * 512],
                            lhsT=kts[i][:, kt * 128 : (kt + 1) * 128],
                            rhs=qts[i][:, qh * 512 : (qh + 1) * 512],
                            start=True,
                            stop=True,
                        )
                    pt = ptp.tile([128, NQ], bf16, tag="pt", name=f"pt{j}_{kt}")
                    nc.scalar.activation(pt, ss, EXP, scale=SCALE)
                    for qh in range(2):
                        nc.tensor.matmul(
                            po[:, qh * 512 : (qh + 1) * 512],
                            lhsT=v1s[i][:, kt, :],
                            rhs=pt[:, qh * 512 : (qh + 1) * 512],
                            start=(kt == 0),
                            stop=(kt == NKT - 1),
                            skip_group_check=True,
                        )
                # normalize: rows 0:64 multiplied by 1/rowsum (row 64)
                zrow = sp.tile([1, NQ], f32, tag="zrow", name=f"zrow{j}")
                nc.vector.tensor_copy(zrow, po[64:65, :])
                recip = sp.tile([1, NQ], f32, tag="recip", name=f"recip{j}")
                nc.vector.reciprocal_approx_fast(out=recip, in_=zrow)
                recip_b = sp.tile([64, NQ], f32, tag="recipb", name=f"recipb{j}")
                nc.gpsimd.partition_broadcast(recip_b, recip)
                nc.vector.tensor_mul(
                    ot_dup[0:64, j * NQ : (j + 1) * NQ], po[0:64, :], recip_b
                )
                # shifted copy for rows 64:128 (ot_dup[64+d, c] = ot_dup[d, c+1])
                lo = j * NQ
                if j == 0:
                    nc.gpsimd.dma_start(
                        out=ot_dup[64:128, 0 : NQ - 1], in_=ot_dup[0:64, 1:NQ]
                    )
                else:
                    nc.gpsimd.dma_start(
                        out=ot_dup[64:128, lo - 1 : lo + NQ - 1],
                        in_=ot_dup[0:64, lo : lo + NQ],
                    )

        # ---- output projection: out[128r + j, :] = sum_u Y_u @ Wp_u + bp ----
        ot3 = ot_dup.rearrange("p (j u) -> p j u", u=12)
        for r in range(4):
            osb = outp.tile([128, C], f32, tag="osb", name=f"osb{r}")
            for n0, n1 in ((0, 384), (384, C)):
                pout = psp.tile([128, 384], f32, tag="proj", name=f"pout{r}_{n0}")
                for t in range(NCT):
                    nc.tensor.matmul(
                        pout,
                        lhsT=ot3[:, r * 128 : (r + 1) * 128, 2 * t],
                        rhs=wp_sb[:, t, n0:n1],
                        start=(t == 0),
                        stop=False,
                        skip_group_check=True,
                    )
                nc.tensor.matmul(
                    pout,
                    lhsT=ones_row,
                    rhs=bp_sb[0:1, n0:n1],
                    start=False,
                    stop=True,
                    skip_group_check=True,
                )
                nc.vector.tensor_copy(osb[:, n0:n1], pout)
            nc.gpsimd.dma_start(out=out[r * 128 : (r + 1) * 128, :], in_=osb)

    if not nc.is_finalized():
        nc.finalize()
    return nc


def make_in_maps(x_q, x_kv, Wq, Wkv, Wp, bp):
    bf = ml_dtypes.bfloat16
    xq_tt = np.ascontiguousarray(
        np.transpose(np.asarray(x_q, np.float32), (0, 2, 1))
    ).astype(bf)  # [B, C, NQ]
    xkv_tt = np.ascontiguousarray(
        np.transpose(np.asarray(x_kv, np.float32), (0, 2, 1))
    ).astype(bf)  # [B, C, NK]
    wq_b = np.asarray(Wq, np.float32).astype(bf)
    wkv_b = np.asarray(Wkv, np.float32).astype(bf)
    wp_b = np.ascontiguousarray(np.asarray(Wp, np.float32).astype(bf))
    bp_b = np.asarray(bp, np.float32).astype(bf).reshape(1, C)

    in_maps = []
    for i in range(8):
        slot_b = [(2 * i + k) % 4 for k in range(4)]
        wqp = np.zeros((4, C, 128), bf)
        wkp = np.zeros((4, C, 128), bf)
        wvp = np.zeros((4, C, 128), bf)
        for s, js in enumerate(SLOT_BLOCKS):
            for idx, j in enumerate(js):
                h = (6 * i + j) // 4
                wqp[s, :, idx * 64 : (idx + 1) * 64] = wq_b[:, h * 64 : (h + 1) * 64]
                wkp[s, :, idx * 64 : (idx + 1) * 64] = wkv_b[:, h * 64 : (h + 1) * 64]
                wvp[s, :, idx * 64 : (idx + 1) * 64] = wkv_b[
                    :, C + h * 64 : C + (h + 1) * 64
                ]
        in_maps.append(
            dict(
                xq_t=np.ascontiguousarray(xq_tt[slot_b]),
                xkv_t=np.ascontiguousarray(xkv_tt[slot_b]),
                wqp=wqp,
                wkp=wkp,
                wvp=wvp,
                wp=wp_b,
                bpv=bp_b,
            )
        )
    return in_maps


def run(inputs, trace=False, **kwargs):
    if "nc" not in _CACHE:
        _CACHE["nc"] = build_nc()
    nc = _CACHE["nc"]
    in_maps = make_in_maps(**inputs)
    res = run_bass_kernel_spmd(nc, in_maps, list(range(8)), trace=trace, **kwargs)
    out = np.concatenate([res.results[i]["out"] for i in range(8)], axis=0)
    return out.reshape(B, NQ, C).astype(np.float32), res


def kernel(x_q, x_kv, Wq, Wkv, Wp, bp):
    out, _ = run(dict(x_q=x_q, x_kv=x_kv, Wq=Wq, Wkv=Wkv, Wp=Wp, bp=bp))
    return out
